# revision 6
# baseline (speedup 1.0000x reference)
"""GCN layer on 8 Trainium2 NeuronCores.

  support = scatter_add(features[src] * w, dst);  out = support @ W.T

Strategy (dst-sharded SPMD, one Bass program for all 8 cores):
  - Core c owns dst rows [c*6250, (c+1)*6250), split into 49 blocks of 128.
  - Edges are routed to the core owning their dst, split per block into
    lo/hi streams by src < 25000 so dma_gather int16 indices stay in range.
  - A shared schedule (per-block chunk counts = max over cores) keeps the
    program identical across cores; edge lists are padded with w=0 dummies.
  - Device pipeline per core:
      dma_gather supers (8K edges) pull feature rows (bf16, padded to 128
      cols = 256B elems) from HBM into SBUF;
      per 128-edge chunk, DVE builds a weighted one-hot
      onehot[e, n] = (n == dst_e) * w_e  (iota + tensor_scalar is_equal/mult);
      PE accumulates supT[d, n] += msgs[e, d].T @ onehot[e, n] into PSUM;
      per block, ACT copies PSUM->SBUF, PE applies W via
      out_blk[n, o] = supT.T @ W.T, ACT copies to an SBUF output buffer;
      one final DMA writes the core's [6250, 64] output slice.
"""
import numpy as np
import ml_dtypes

BF16 = ml_dtypes.bfloat16

N_NODES = 50000
N_CORES = 8
D_IN = 64
D_OUT = 64
SPLIT = 25000          # int16 gather index range guard
BLOCK = 128            # dst rows per block
CHUNK = 128            # edges per matmul chunk
SUPER_CHUNKS = 64      # chunks per dma_gather call (8192 edges)
NODES_PER_CORE = N_NODES // N_CORES           # 6250
N_BLOCKS = -(-NODES_PER_CORE // BLOCK)        # 49


# ---------------------------------------------------------------- host prep

def _build_core_data(edge_src, edge_dst, edge_w):
    """Shared schedule + per-core padded edge arrays."""
    core_of_edge = edge_dst // NODES_PER_CORE
    per_core_lists = []
    for c in range(N_CORES):
        e_idx = np.nonzero(core_of_edge == c)[0]
        dst_local = edge_dst[e_idx] - c * NODES_PER_CORE
        blk = dst_local // BLOCK
        lo = edge_src[e_idx] < SPLIT
        per_core_lists.append([
            (e_idx[(blk == k) & lo], e_idx[(blk == k) & ~lo])
            for k in range(N_BLOCKS)
        ])

    K_lo = np.zeros(N_BLOCKS, dtype=np.int64)
    K_hi = np.zeros(N_BLOCKS, dtype=np.int64)
    for k in range(N_BLOCKS):
        for c in range(N_CORES):
            lo_e, hi_e = per_core_lists[c][k]
            K_lo[k] = max(K_lo[k], -(-len(lo_e) // CHUNK))
            K_hi[k] = max(K_hi[k], -(-len(hi_e) // CHUNK))

    T_lo, T_hi = int(K_lo.sum()), int(K_hi.sum())
    T = T_lo + T_hi

    sched = []  # chunk t -> (block, half, pos_in_stream)
    p = [0, 0]
    for k in range(N_BLOCKS):
        for half, K in ((0, int(K_lo[k])), (1, int(K_hi[k]))):
            for _ in range(K):
                sched.append((k, half, p[half]))
                p[half] += 1

    cores = []
    for c in range(N_CORES):
        idx_stream = [np.zeros(T_lo * CHUNK, dtype=np.int16),
                      np.zeros(T_hi * CHUNK, dtype=np.int16)]
        dst_t = np.zeros((CHUNK, T), dtype=np.float32)
        w_t = np.zeros((CHUNK, T), dtype=np.float32)
        pos = [0, 0]
        t = 0
        for k in range(N_BLOCKS):
            lo_e, hi_e = per_core_lists[c][k]
            for half, (e_list, K) in enumerate(((lo_e, K_lo[k]), (hi_e, K_hi[k]))):
                K = int(K)
                n = len(e_list)
                padded = K * CHUNK
                buf_i = np.zeros(padded, dtype=np.int16)
                buf_d = np.zeros(padded, dtype=np.float32)
                buf_w = np.zeros(padded, dtype=np.float32)
                buf_i[:n] = (edge_src[e_list] - half * SPLIT).astype(np.int16)
                buf_d[:n] = (edge_dst[e_list] - c * NODES_PER_CORE
                             - k * BLOCK).astype(np.float32)
                buf_w[:n] = edge_w[e_list].astype(np.float32)
                s = pos[half] * CHUNK
                idx_stream[half][s:s + padded] = buf_i
                dst_t[:, t:t + K] = buf_d.reshape(K, CHUNK).T
                w_t[:, t:t + K] = buf_w.reshape(K, CHUNK).T
                pos[half] += K
                t += K

        def wrap(flat):
            # dma_gather index layout: edge j at [j%16, j//16], replicated to
            # the 8 groups of 16 partitions (one per Q7 core).
            cols = max(len(flat) // 16, 1)
            w16 = flat.reshape(cols, 16).T if len(flat) else np.zeros(
                (16, 1), dtype=np.int16)
            return np.ascontiguousarray(np.tile(w16, (8, 1)))

        cores.append(dict(idx_lo=wrap(idx_stream[0]),
                          idx_hi=wrap(idx_stream[1]),
                          dst_t=np.ascontiguousarray(dst_t),
                          w_t=np.ascontiguousarray(w_t)))

    shared = dict(K_lo=tuple(int(x) for x in K_lo),
                  K_hi=tuple(int(x) for x in K_hi),
                  T_lo=T_lo, T_hi=T_hi, T=T, sched=tuple(sched))
    return shared, cores


# ------------------------------------------------------------- bass program

def _build_program(shared):
    import concourse.bacc as bacc
    import concourse.tile as tile
    import concourse.mybir as mybir

    f32 = mybir.dt.float32
    bf16 = mybir.dt.bfloat16
    i16 = mybir.dt.int16

    T_lo, T_hi, T = shared["T_lo"], shared["T_hi"], shared["T"]
    sched = shared["sched"]

    nc = bacc.Bacc("TRN2", target_bir_lowering=False, debug=False,
                   num_devices=N_CORES)

    feats = nc.dram_tensor("feats", [N_NODES, 128], bf16, kind="ExternalInput")
    idx_lo = nc.dram_tensor("idx_lo", [128, max(T_lo * 8, 1)], i16,
                            kind="ExternalInput")
    idx_hi = nc.dram_tensor("idx_hi", [128, max(T_hi * 8, 1)], i16,
                            kind="ExternalInput")
    dst_t = nc.dram_tensor("dst_t", [128, T], f32, kind="ExternalInput")
    w_t = nc.dram_tensor("w_t", [128, T], f32, kind="ExternalInput")
    w_T = nc.dram_tensor("w_T", [D_IN, D_OUT], f32, kind="ExternalInput")
    out = nc.dram_tensor("out", [NODES_PER_CORE, D_OUT], f32,
                         kind="ExternalOutput")

    n_sup = [-(-T_lo // SUPER_CHUNKS), -(-T_hi // SUPER_CHUNKS)]
    stream_T = [T_lo, T_hi]
    idx_dram = [idx_lo, idx_hi]
    feats_half = [feats[0:SPLIT, :], feats[SPLIT:N_NODES, :]]

    with tile.TileContext(nc) as tc:
        with (
            tc.tile_pool(name="const", bufs=1) as cpool,
            tc.tile_pool(name="glo", bufs=3) as glo_pool,
            tc.tile_pool(name="ghi", bufs=3) as ghi_pool,
            tc.tile_pool(name="oh", bufs=8) as oh_pool,
            tc.tile_pool(name="sup_sb", bufs=3) as sup_sb_pool,
            tc.tile_pool(name="psum_acc", bufs=4, space="PSUM") as acc_pool,
            tc.tile_pool(name="psum_out", bufs=2, space="PSUM") as opsum_pool,
        ):
            # resident inputs
            idx_sb = []
            for half in range(2):
                tl = cpool.tile([128, max(stream_T[half] * 8, 1)], i16,
                                tag=f"idx{half}")
                nc.sync.dma_start(tl[:], idx_dram[half][:])
                idx_sb.append(tl)
            dst_sb = cpool.tile([128, T], f32, tag="dst")
            nc.sync.dma_start(dst_sb[:], dst_t[:])
            w_sb = cpool.tile([128, T], f32, tag="w")
            nc.sync.dma_start(w_sb[:], w_t[:])
            wT_sb = cpool.tile([D_IN, D_OUT], f32, tag="wT")
            nc.sync.dma_start(wT_sb[:], w_T[:])
            iota_t = cpool.tile([128, BLOCK], bf16, tag="iota")
            nc.gpsimd.iota(iota_t[:], [[1, BLOCK]], channel_multiplier=0,
                           allow_small_or_imprecise_dtypes=True)
            out_sb = cpool.tile([128, N_BLOCKS, D_OUT], f32, tag="outsb")

            gather_tiles = [{}, {}]
            gpools = [glo_pool, ghi_pool]

            def ensure_super(half, s):
                if s in gather_tiles[half]:
                    return gather_tiles[half][s]
                g_chunks = min(SUPER_CHUNKS, stream_T[half] - s * SUPER_CHUNKS)
                n_idx = g_chunks * CHUNK
                gt = gpools[half].tile([128, g_chunks, 128], bf16,
                                       tag=f"g{half}")
                c0 = s * SUPER_CHUNKS * 8
                nc.gpsimd.dma_gather(
                    gt[:],
                    feats_half[half],
                    idx_sb[half][:, c0:c0 + n_idx // 16],
                    n_idx,
                    n_idx,
                    elem_size=128,
                    single_packet=False,
                )
                gather_tiles[half][s] = gt
                return gt

            # chunk stream
            acc = None
            for t, (k, half, p) in enumerate(sched):
                first = t == 0 or sched[t - 1][0] != k
                last = t == len(sched) - 1 or sched[t + 1][0] != k
                if first:
                    acc = acc_pool.tile([D_IN, BLOCK], f32, tag="acc")
                gt = ensure_super(half, p // SUPER_CHUNKS)
                g = p % SUPER_CHUNKS
                oh = oh_pool.tile([128, BLOCK], bf16, tag="oh")
                nc.vector.tensor_scalar(
                    oh[:], iota_t[:],
                    dst_sb[:, t:t + 1], w_sb[:, t:t + 1],
                    mybir.AluOpType.is_equal, mybir.AluOpType.mult,
                )
                nc.tensor.matmul(
                    acc[:], gt[:, g, 0:D_IN], oh[:],
                    start=first, stop=last,
                )
                if last:
                    sup_sb = sup_sb_pool.tile([D_IN, BLOCK], f32, tag="sup")
                    nc.scalar.copy(sup_sb[:], acc[:])
                    ob = opsum_pool.tile([BLOCK, D_OUT], f32, tag="ob")
                    nc.tensor.matmul(ob[:], sup_sb[:], wT_sb[:],
                                     start=True, stop=True)
                    nc.scalar.copy(out_sb[:, k, :], ob[:])

            # write out: 48 full blocks + 106-row tail
            n_full = NODES_PER_CORE // BLOCK
            rem = NODES_PER_CORE - n_full * BLOCK
            nc.sync.dma_start(
                out[0:n_full * BLOCK, :].rearrange("(k p) d -> p k d", p=BLOCK),
                out_sb[:, 0:n_full, :],
            )
            if rem:
                nc.sync.dma_start(
                    out[n_full * BLOCK:NODES_PER_CORE, :],
                    out_sb[0:rem, n_full, :],
                )

    nc.compile()
    return nc


# --------------------------------------------------------------------- run

_CACHE = {}


def _get_program(shared):
    key = (shared["K_lo"], shared["K_hi"])
    if key not in _CACHE:
        _CACHE[key] = _build_program(shared)
    return _CACHE[key]


LAST_EXEC_NS = None


def kernel(features, edge_src, edge_dst, edge_w, weight):
    import os
    global LAST_EXEC_NS
    from concourse.bass_utils import run_bass_kernel_spmd

    features = np.asarray(features, dtype=np.float32)
    edge_src = np.asarray(edge_src).astype(np.int64)
    edge_dst = np.asarray(edge_dst).astype(np.int64)
    edge_w = np.asarray(edge_w, dtype=np.float32)
    weight = np.asarray(weight, dtype=np.float32)

    shared, cores = _build_core_data(edge_src, edge_dst, edge_w)
    nc = _get_program(shared)

    feats_pad = np.zeros((N_NODES, 128), dtype=BF16)
    feats_pad[:, :D_IN] = features.astype(BF16)
    w_T = np.ascontiguousarray(weight.T)

    in_maps = [
        dict(feats=feats_pad, idx_lo=cores[c]["idx_lo"],
             idx_hi=cores[c]["idx_hi"], dst_t=cores[c]["dst_t"],
             w_t=cores[c]["w_t"], w_T=w_T)
        for c in range(N_CORES)
    ]
    trace = os.environ.get("GCN_TRACE", "") == "1"
    res = run_bass_kernel_spmd(nc, in_maps, core_ids=list(range(N_CORES)),
                               trace=trace)
    if res.exec_time_ns is not None:
        LAST_EXEC_NS = res.exec_time_ns
    return np.concatenate([r["out"] for r in res.results], axis=0)
